# revision 19
# baseline (speedup 1.0000x reference)
"""MD-LSTM (4-direction 2D LSTM) Trainium2 Bass kernel.

Sharding (8 NeuronCores, SPMD): core c handles direction (c % 4) with batch
half (c // 4); the 16-batch half is split into TWO interleaved sub-scans of 8
(A, B).  The two sub-scans are independent recurrences: the tensor engine
runs B's matmuls while A's pointwise tail executes (and vice versa).

Per sub-scan the H,W recurrence runs as anti-diagonal wavefronts: 159 steps,
gates for the valid diagonal cells (<=32) x 8 batch = <=256 matmul columns,
contracting [x(64); 1; 1; h_up(128); h_lf(128)] against [w0; b_hi; b_lo;
u0; u1] (fp16) into PSUM, then the LSTM cell update on ACT/DVE/Pool with
fp16 c-state.

Key scheduling:
- Bias rides the x-projection as two ones-rows (fp16 bias + fp16 residual,
  K=66) so activations need no per-gate bias and the bias is fp32-accurate.
- PSUM: one 2KB bank == one accumulation group.  l,f packed into one bank,
  i,o,g into a 2-bank pair; only the first matmul per bank has start=True.
- sigmoid-everywhere: the g-gate weights are pre-scaled x2 on the host so
  tanh(g) == 2*sigmoid(2g) - 1.  One sigmoid covers the whole i/o/g pair
  (8 -> 6 ACT ops per step); the 2s-1 fixup is a fused dual-op
  tensor_scalar on DVE.
- u-matmul order l,f,g,i,o: sigmoid(l,f) fires while the PE is still on
  g,i,o; the DVE c-chain starts early.
- pointwise is split: early (sig_lf, sig_iog, gg, ig, c-chain) issued
  right after the sub-scan's matmuls; late (tanh_c, h-mult) issued AFTER the
  other sub-scan's matmuls+early block, so the in-order ACT/DVE queues never
  head-of-line-block the other sub-scan's chain.
- h state lives in a 32-slot ring (slot = step % 32) that doubles as the
  output staging buffer: one 16-step chunk DMA out instead of per-step DMAs;
  x is prefetched in 16-step chunks into a 32-slot ring likewise.

State: h (fp16) ring (OC, 32*(8 guard + 256)); c (fp16) double-buffered
(OC, 8 + 256).  Column = guard + y*8 + b; up-neighbor = column offset -8 in
the previous step's slot; guard stays zero; stale slots are never read.

Self-contained: hardcodes all shapes; reads no files.
"""
import numpy as np

import concourse.bass as bass
import concourse.bacc as bacc
import concourse.mybir as mybir
import concourse.tile as tile
from concourse import bass_utils

B, CIN, H, W, OC = 32, 64, 32, 128, 128
CINB = CIN + 2              # +2 ones-rows carrying bias hi/lo
NSTEP = H + W - 1           # 159
BQ = 8                      # batch per sub-scan
SWQ = H * BQ                # 256 max window cells
HWQ = BQ + SWQ              # guard + slots = 264
RING = 32                   # ring slots (2 chunks of CHUNK)
CHUNK = 16                  # steps per DMA chunk
FP = mybir.dt.float32
HF = mybir.dt.float16

# host-side gate reorder: [l, f, i, o, g] (reference order [i, f, g, o, l])
GATE_ORDER = [4, 1, 0, 3, 2]
J_L, J_F, J_I, J_O, J_G = 0, 1, 2, 3, 4


def _window(d):
    return max(0, d - (W - 1)), min(d, H - 1)


def build_kernel():
    nc = bacc.Bacc("TRN2", target_bir_lowering=False, debug=False, num_devices=8)

    xs_d = [nc.dram_tensor(f"x_diag{s}", [CINB, NSTEP * SWQ], HF,
                           kind="ExternalInput") for s in range(2)]
    w0_d = nc.dram_tensor("w0", [CINB, 5 * OC], FP, kind="ExternalInput")
    u0_d = nc.dram_tensor("u0", [OC, 5 * OC], FP, kind="ExternalInput")
    u1_d = nc.dram_tensor("u1", [OC, 5 * OC], FP, kind="ExternalInput")
    outs_d = [nc.dram_tensor(f"out_diag{s}", [OC, NSTEP * HWQ], HF,
                             kind="ExternalOutput") for s in range(2)]

    act = mybir.ActivationFunctionType
    alu = mybir.AluOpType

    with tile.TileContext(nc) as tc:
        with (
            tc.tile_pool(name="const", bufs=1) as cpool,
            tc.tile_pool(name="state", bufs=1) as spool,
            tc.tile_pool(name="gates", bufs=6) as gpool,
            tc.tile_pool(name="tmp", bufs=6) as tpool,
            tc.tile_pool(name="psum", bufs=3, space="PSUM") as ppool,
        ):
            # ---- weights (fp32 load -> bf16 cast once) ----
            w0s = cpool.tile([CINB, 5 * OC], FP, tag="w0")
            u0s = cpool.tile([OC, 5 * OC], FP, tag="u0")
            u1s = cpool.tile([OC, 5 * OC], FP, tag="u1")
            nc.sync.dma_start(w0s[:, :], w0_d.ap())
            nc.sync.dma_start(u0s[:, :], u0_d.ap())
            nc.sync.dma_start(u1s[:, :], u1_d.ap())
            w0m = cpool.tile([CINB, 5 * OC], HF, tag="w0b")
            u0m = cpool.tile([OC, 5 * OC], HF, tag="u0b")
            u1m = cpool.tile([OC, 5 * OC], HF, tag="u1b")
            nc.vector.tensor_copy(w0m[:, :], w0s[:, :])
            nc.vector.tensor_copy(u0m[:, :], u0s[:, :])
            nc.vector.tensor_copy(u1m[:, :], u1s[:, :])

            # ---- rings and state ----
            # x ring: 32 slots of SWQ cols; h ring: 32 slots of HWQ cols
            xr = [spool.tile([CINB, RING * SWQ], HF, tag=f"xr{s}",
                             name=f"xr{s}") for s in range(2)]
            hr = [spool.tile([OC, RING * HWQ], HF, tag=f"hr{s}",
                             name=f"hr{s}") for s in range(2)]
            cb = [[spool.tile([OC, HWQ], HF, tag=f"cb{s}{k}", name=f"cb{s}{k}")
                   for k in range(3)] for s in range(2)]
            for s in range(2):
                nc.vector.memset(hr[s][:, :], 0.0)
                for k in range(3):
                    nc.vector.memset(cb[s][k][:, :], 0.0)

            def load_x_chunk(s, c):
                """DMA x chunk c (steps [16c, 16c+16)) into its ring half."""
                d0 = c * CHUNK
                n = min(CHUNK, NSTEP - d0) * SWQ
                nc.sync.dma_start(
                    xr[s][:, (d0 % RING) * SWQ: (d0 % RING) * SWQ + n],
                    xs_d[s].ap()[:, d0 * SWQ: d0 * SWQ + n])

            def store_h_chunk(s, c):
                """DMA h chunk c (steps [16c, 16c+16)) from its ring half."""
                d0 = c * CHUNK
                n = min(CHUNK, NSTEP - d0) * HWQ
                nc.sync.dma_start(
                    outs_d[s].ap()[:, d0 * HWQ: d0 * HWQ + n],
                    hr[s][:, (d0 % RING) * HWQ: (d0 % RING) * HWQ + n])

            load_x_chunk(0, 0)
            load_x_chunk(1, 0)
            load_x_chunk(0, 1)
            load_x_chunk(1, 1)

            class Step:
                pass

            XORD = (J_L, J_F, J_G, J_I, J_O)
            UORD = (J_L, J_F, J_G, J_I, J_O)

            def mm_x_lf(s, d, st):
                """x-projection into the l,f bank (issued at step top; the
                lf bank's only reader sig_lf(d-1) finished early)."""
                y0, y1 = _window(d)
                nwin = (y1 - y0 + 1) * BQ
                st.y0, st.nwin = y0, nwin
                st.lo = BQ + y0 * BQ
                st.xv = xr[s][:, (d % RING) * SWQ + y0 * BQ:
                              (d % RING) * SWQ + y0 * BQ + nwin]
                # PSUM: one 2KB bank == one accumulation group (start=True
                # zeroes the whole bank).  l,f in one bank; i,o,g in a
                # 2-bank pair so ONE sigmoid covers all three.
                st.lf = ppool.tile([OC, 2 * SWQ], FP, tag="lf", bufs=2,
                                   name=f"lf{s}_{d}")
                st.iog = ppool.tile([OC, 3 * SWQ], FP, tag="iog", bufs=2,
                                    name=f"iog{s}_{d}")
                # dense i@0,o@n,g@2n unless g would straddle the bank
                # boundary (2n < 512 < 3n); then slot i@0,o@256,g@512
                st.dense = not (2 * nwin < 512 < 3 * nwin)
                st.gstr = nwin if st.dense else SWQ
                iogslot = {J_I: 0, J_O: 1, J_G: 2}

                def dst(j):
                    if j in (J_L, J_F):
                        return st.lf[:, (j - J_L) * nwin:(j - J_L + 1) * nwin]
                    k = iogslot[j] * st.gstr
                    return st.iog[:, k:k + nwin]

                st.dst = dst
                for j in (J_L, J_F):
                    nc.tensor.matmul(dst(j),
                                     w0m[:, j * OC:(j + 1) * OC], st.xv,
                                     start=(j == J_L), stop=False)

            def mm_x_iog(s, d, st):
                """x-projection into the i,o,g bank pair, issued right
                before this sub-scan's u-matmuls: by then the pair's
                reader sig_iog(d-1) is long done, so the WAR wait that
                would otherwise head-of-line-block the PE FIFO is free."""
                nwin = st.nwin
                # start=True on the first matmul into each bank: g (iog
                # bank 0 when all three pack into it, else bank 1), i
                # (iog bank 0) when g sits in bank 1.
                starts = (J_G,) if 3 * nwin <= 512 else (J_G, J_I)
                for j in (J_G, J_I, J_O):
                    nc.tensor.matmul(st.dst(j),
                                     w0m[:, j * OC:(j + 1) * OC], st.xv,
                                     start=(j in starts), stop=False)

            def mm_u(s, d, st):
                """h-recurrent matmuls for sub-scan s at step d."""
                nwin, lo = st.nwin, st.lo
                pbase = ((d - 1) % RING) * HWQ
                rhs_up = hr[s][:, pbase + lo - BQ: pbase + lo - BQ + nwin]
                rhs_lf = hr[s][:, pbase + lo: pbase + lo + nwin]
                stops = (J_F, J_O) if 3 * nwin <= 512 else (J_F, J_O, J_G)
                for j in UORD:
                    nc.tensor.matmul(st.dst(j),
                                     u0m[:, j * OC:(j + 1) * OC], rhs_up,
                                     start=False, stop=False)
                    nc.tensor.matmul(st.dst(j),
                                     u1m[:, j * OC:(j + 1) * OC], rhs_lf,
                                     start=False, stop=(j in stops))

            def dcx_pre(s, d, st, prev):
                """dcx = c_up - c_lf: only needs the previous step's c (Pool,
                off the critical path)."""
                nwin, lo = st.nwin, st.lo
                c_up = cb[s][prev][:, lo - BQ: lo - BQ + nwin]
                c_lf = cb[s][prev][:, lo: lo + nwin]
                dcx = tpool.tile([OC, SWQ], HF, tag=f"dcx{s}", name=f"dcx{s}_{d}")
                nc.gpsimd.tensor_tensor(dcx[:, 0:nwin], c_up, c_lf, alu.subtract)
                st.dcx = dcx

            def sig_lf(s, d, st):
                """sigmoid over the l,f bank (fires after 4 u-matmuls)."""
                nwin = st.nwin
                g4 = gpool.tile([OC, 2 * SWQ], HF, tag=f"g4{s}", name=f"g4{s}_{d}")
                st.g4 = g4
                nc.scalar.activation(g4[:, 0:2 * nwin],
                                     st.lf[:, 0:2 * nwin], act.Sigmoid)

            def sig_iog(s, d, st):
                """one sigmoid over i,o,g (g pre-scaled x2 on host; the
                slotted layout activates the garbage gap cols too)."""
                nwin, gstr = st.nwin, st.gstr
                gio = gpool.tile([OC, 3 * SWQ], HF, tag=f"gio{s}",
                                 name=f"gio{s}_{d}")
                st.gio = gio
                span = 2 * gstr + nwin
                nc.scalar.activation(gio[:, 0:span], st.iog[:, 0:span],
                                     act.Sigmoid)

            def pw_mix(s, d, st, prev):
                """the l,f half of the c-chain (only needs sig_lf)."""
                nwin, lo = st.nwin, st.lo
                c_lf = cb[s][prev][:, lo: lo + nwin]
                g4 = st.g4
                l_ = g4[:, 0 * nwin:1 * nwin]
                f_ = g4[:, 1 * nwin:2 * nwin]
                mix = tpool.tile([OC, SWQ], HF, tag=f"mix{s}", name=f"mix{s}_{d}")
                nc.vector.tensor_tensor(mix[:, 0:nwin], l_, st.dcx[:, 0:nwin],
                                        alu.mult)
                nc.vector.tensor_tensor(mix[:, 0:nwin], mix[:, 0:nwin], c_lf,
                                        alu.add)
                nc.vector.tensor_tensor(mix[:, 0:nwin], f_, mix[:, 0:nwin],
                                        alu.mult)
                st.mix = mix

            def pw_ig(s, d, st, cur):
                """i*g and the c write (needs sig_iog)."""
                nwin, lo, gstr = st.nwin, st.lo, st.gstr
                i_ = st.gio[:, 0:nwin]
                sg = st.gio[:, 2 * gstr:2 * gstr + nwin]
                # tanh(g) = 2*sigmoid(2g) - 1, fused dual-op tensor_scalar
                gg = tpool.tile([OC, SWQ], HF, tag=f"gg{s}", name=f"gg{s}_{d}")
                nc.vector.tensor_scalar(gg[:, 0:nwin], sg, 2.0, 1.0,
                                        alu.mult, alu.subtract)
                ig = tpool.tile([OC, SWQ], HF, tag=f"ig{s}", name=f"ig{s}_{d}")
                nc.vector.tensor_tensor(ig[:, 0:nwin], i_, gg[:, 0:nwin], alu.mult)
                cw = cb[s][cur][:, lo: lo + nwin]
                nc.vector.tensor_tensor(cw, st.mix[:, 0:nwin], ig[:, 0:nwin],
                                        alu.add)
                st.cw = cw

            def tanh_c(s, d, st):
                th = tpool.tile([OC, SWQ], HF, tag=f"th{s}", name=f"th{s}_{d}")
                nc.scalar.activation(th[:, 0:st.nwin], st.cw, act.Tanh)
                st.th = th

            def h_mul(s, d, st):
                nwin, lo, gstr = st.nwin, st.lo, st.gstr
                o_ = st.gio[:, gstr:gstr + nwin]
                base = (d % RING) * HWQ
                hwv = hr[s][:, base + lo: base + lo + nwin]
                nc.vector.tensor_tensor(hwv, o_, st.th[:, 0:nwin], alu.mult)

            scr = ppool.tile([OC, 512], FP, tag="scratch", bufs=1,
                             name="scratch")

            def pe_fill(k):
                """Dummy matmuls on constant tiles: no deps, keep the PE
                clock ramped while it waits for the h chain."""
                for _ in range(k):
                    nc.tensor.matmul(scr[:, 0:512], u1m[:, 0:OC],
                                     u0m[:, 0:512], start=True, stop=True)

            for d in range(NSTEP):
                cur, prev = d % 3, (d + 2) % 3
                stA, stB = Step(), Step()
                mm_x_lf(0, d, stA)
                mm_x_lf(1, d, stB)
                dcx_pre(0, d, stA, prev)
                dcx_pre(1, d, stB, prev)
                pe_fill(4)
                mm_x_iog(1, d, stB)
                mm_u(1, d, stB)
                sig_lf(1, d, stB)
                if d > 0:
                    # A's tail from step d-1, deferred: its tanh/h slot into
                    # the ACT/DVE queues here (deps long met) instead of
                    # head-of-line blocking this step's B burst
                    tanh_c(0, d - 1, stA_prev)
                    h_mul(0, d - 1, stA_prev)
                    if (d - 1) % CHUNK == CHUNK - 1:
                        # A's chunk store waits for the deferred h above
                        store_h_chunk(0, (d - 1) // CHUNK)
                sig_iog(1, d, stB)
                pw_mix(1, d, stB, prev)
                pw_ig(1, d, stB, cur)
                mm_x_iog(0, d, stA)
                mm_u(0, d, stA)
                pe_fill(1)
                tanh_c(1, d, stB)         # ACT: B's tanh ahead of A's sigmoids
                sig_lf(0, d, stA)
                sig_iog(0, d, stA)
                pw_mix(0, d, stA, prev)
                pw_ig(0, d, stA, cur)
                h_mul(1, d, stB)
                stA_prev = stA
                if d % CHUNK == CHUNK - 1:
                    c = d // CHUNK
                    # B's h chunk done -> store (A's ships next step, after
                    # its deferred tail); prefetch x chunk c+2
                    store_h_chunk(1, c)
                    if (c + 2) * CHUNK < NSTEP:
                        load_x_chunk(0, c + 2)
                        load_x_chunk(1, c + 2)
            # flush A's deferred tail from the last step
            tanh_c(0, NSTEP - 1, stA_prev)
            h_mul(0, NSTEP - 1, stA_prev)
            # tail: steps 144..158 are chunk 9 (15 steps); A's chunk 8
            # store was deferred into step 144 above
            store_h_chunk(0, NSTEP // CHUNK)
            store_h_chunk(1, NSTEP // CHUNK)

    nc.compile()
    return nc


_NC_CACHE = {}


def _get_nc():
    if "nc" not in _NC_CACHE:
        _NC_CACHE["nc"] = build_kernel()
    return _NC_CACHE["nc"]


def _flip(x, d):
    if d == 1:
        return x[:, :, :, ::-1]
    if d == 2:
        return x[:, :, ::-1, :]
    if d == 3:
        return x[:, :, ::-1, ::-1]
    return x


def _make_x_diag(x_nat):
    """(CIN,H,W,BQ) -> (CINB, NSTEP*SWQ) diagonal layout with ones-rows."""
    arr = np.zeros((CINB, NSTEP, H, BQ), np.float32)
    for y in range(H):
        arr[:CIN, y:y + W, y, :] = x_nat[:, y, :, :]
    arr[CIN:, :, :, :] = 1.0
    return arr.reshape(CINB, NSTEP * SWQ)


def _decode(out_diag):
    """(OC, NSTEP*HWQ) fp32 -> (BQ, OC, H, W); skip the guard columns."""
    arr = out_diag.reshape(OC, NSTEP, HWQ)[:, :, BQ:].reshape(OC, NSTEP, H, BQ)
    out = np.empty((BQ, OC, H, W), np.float32)
    for y in range(H):
        out[:, :, y, :] = arr[:, y:y + W, y, :].transpose(2, 0, 1)
    return out


def kernel(x, w0, u0, u1, b, trace=False, _res=[None]):
    x = np.asarray(x, np.float32)
    w0 = np.asarray(w0, np.float32)
    u0 = np.asarray(u0, np.float32)
    u1 = np.asarray(u1, np.float32)
    b = np.asarray(b, np.float32)

    perm = np.concatenate([np.arange(g * OC, (g + 1) * OC) for g in GATE_ORDER])
    in_maps = []
    for c in range(8):
        dirn, half = c % 4, c // 4
        xs = _flip(x[half * 16:(half + 1) * 16], dirn)          # (16,CIN,H,W)
        x_nat = np.ascontiguousarray(xs.transpose(1, 2, 3, 0))  # (CIN,H,W,16)
        # g-gate pre-scale: tanh(g) = 2*sigmoid(2g) - 1, so double the g
        # columns of every projection (and bias) and apply sigmoid on-chip
        w0p = w0[dirn][:, perm].copy()
        u0p = u0[dirn][:, perm].copy()
        u1p = u1[dirn][:, perm].copy()
        bp = b[dirn][perm].copy()
        w0p[:, J_G * OC:(J_G + 1) * OC] *= 2.0
        u0p[:, J_G * OC:(J_G + 1) * OC] *= 2.0
        u1p[:, J_G * OC:(J_G + 1) * OC] *= 2.0
        bp[J_G * OC:(J_G + 1) * OC] *= 2.0
        # bias split into fp16 hi + residual lo rows so it lands fp32-accurate
        b_hi = bp.astype(np.float16).astype(np.float32)
        b_lo = bp - b_hi
        w0b = np.concatenate([w0p, b_hi[None, :], b_lo[None, :]], axis=0)
        m = {
            "w0": np.ascontiguousarray(w0b),
            "u0": np.ascontiguousarray(u0p),
            "u1": np.ascontiguousarray(u1p),
        }
        for s in range(2):
            m[f"x_diag{s}"] = _make_x_diag(
                x_nat[:, :, :, s * BQ:(s + 1) * BQ]).astype(np.float16)
        in_maps.append(m)

    nc = _get_nc()
    res = bass_utils.run_bass_kernel_spmd(nc, in_maps, list(range(8)), trace=trace)
    _res[0] = res

    out = np.empty((B, 4, OC, H, W), np.float32)
    for c in range(8):
        dirn, half = c % 4, c // 4
        for s in range(2):
            od = np.asarray(res.results[c][f"out_diag{s}"]).astype(np.float32)
            lo = half * 16 + s * BQ
            out[lo:lo + BQ, dirn] = _decode(od)
    return out



# revision 26
# speedup vs baseline: 1.0132x; 1.0132x over previous
"""MD-LSTM (4-direction 2D LSTM) Trainium2 Bass kernel.

Sharding (8 NeuronCores, SPMD): core c handles direction (c % 4) with batch
half (c // 4); the 16-batch half is split into TWO interleaved sub-scans of 8
(A, B).  The two sub-scans are independent recurrences: the tensor engine
runs B's matmuls while A's pointwise tail executes (and vice versa).

Per sub-scan the H,W recurrence runs as anti-diagonal wavefronts: 159 steps,
gates for the valid diagonal cells (<=32) x 8 batch = <=256 matmul columns,
contracting [x(64); 1; 1; h_up(128); h_lf(128)] against [w0; b_hi; b_lo;
u0; u1] (fp16) into PSUM, then the LSTM cell update on ACT/DVE/Pool with
fp16 c-state.

Key scheduling:
- Bias rides the x-projection as two ones-rows (fp16 bias + fp16 residual,
  K=66) so activations need no per-gate bias and the bias is fp32-accurate.
- PSUM: one 2KB bank == one accumulation group.  l,f packed into one bank,
  i,o,g into a 2-bank pair; only the first matmul per bank has start=True.
- sigmoid-everywhere: the g-gate weights are pre-scaled x2 on the host so
  tanh(g) == 2*sigmoid(2g) - 1.  One sigmoid covers the whole i/o/g pair
  (8 -> 6 ACT ops per step); the 2s-1 fixup is a fused dual-op
  tensor_scalar on DVE.
- u-matmul order l,f,g,i,o: sigmoid(l,f) fires while the PE is still on
  g,i,o; the DVE c-chain starts early.
- pointwise is split: early (sig_lf, sig_iog, gg, ig, c-chain) issued
  right after the sub-scan's matmuls; late (tanh_c, h-mult) issued AFTER the
  other sub-scan's matmuls+early block, so the in-order ACT/DVE queues never
  head-of-line-block the other sub-scan's chain.
- h state lives in a 32-slot ring (slot = step % 32) that doubles as the
  output staging buffer: one 16-step chunk DMA out instead of per-step DMAs;
  x is prefetched in 16-step chunks into a 32-slot ring likewise.

State: h (fp16) ring (OC, 32*(8 guard + 256)); c (fp16) double-buffered
(OC, 8 + 256).  Column = guard + y*8 + b; up-neighbor = column offset -8 in
the previous step's slot; guard stays zero; stale slots are never read.

Self-contained: hardcodes all shapes; reads no files.
"""
import numpy as np

import concourse.bass as bass
import concourse.bacc as bacc
import concourse.mybir as mybir
import concourse.tile as tile
from concourse import bass_utils

B, CIN, H, W, OC = 32, 64, 32, 128, 128
CINB = CIN + 2              # +2 ones-rows carrying bias hi/lo
NSTEP = H + W - 1           # 159
BQ = 8                      # batch per sub-scan
SWQ = H * BQ                # 256 max window cells
HWQ = BQ + SWQ              # guard + slots = 264
RING = 32                   # ring slots (2 chunks of CHUNK)
CHUNK = 16                  # steps per DMA chunk
FP = mybir.dt.float32
HF = mybir.dt.float16

# host-side gate reorder: [l, f, i, o, g] (reference order [i, f, g, o, l])
GATE_ORDER = [4, 1, 0, 3, 2]
J_L, J_F, J_I, J_O, J_G = 0, 1, 2, 3, 4


def _window(d):
    return max(0, d - (W - 1)), min(d, H - 1)


def build_kernel():
    nc = bacc.Bacc("TRN2", target_bir_lowering=False, debug=False, num_devices=8)

    xs_d = [nc.dram_tensor(f"x_diag{s}", [CINB, NSTEP * SWQ], HF,
                           kind="ExternalInput") for s in range(2)]
    w0_d = nc.dram_tensor("w0", [CINB, 5 * OC], FP, kind="ExternalInput")
    u0_d = nc.dram_tensor("u0", [OC, 5 * OC], FP, kind="ExternalInput")
    u1_d = nc.dram_tensor("u1", [OC, 5 * OC], FP, kind="ExternalInput")
    outs_d = [nc.dram_tensor(f"out_diag{s}", [OC, NSTEP * HWQ], HF,
                             kind="ExternalOutput") for s in range(2)]

    act = mybir.ActivationFunctionType
    alu = mybir.AluOpType

    with tile.TileContext(nc) as tc:
        with (
            tc.tile_pool(name="const", bufs=1) as cpool,
            tc.tile_pool(name="state", bufs=1) as spool,
            tc.tile_pool(name="gates", bufs=6) as gpool,
            tc.tile_pool(name="tmp", bufs=6) as tpool,
            tc.tile_pool(name="psum", bufs=3, space="PSUM") as ppool,
        ):
            # ---- weights (fp32 load -> bf16 cast once) ----
            w0s = cpool.tile([CINB, 5 * OC], FP, tag="w0")
            u0s = cpool.tile([OC, 5 * OC], FP, tag="u0")
            u1s = cpool.tile([OC, 5 * OC], FP, tag="u1")
            nc.sync.dma_start(w0s[:, :], w0_d.ap())
            nc.sync.dma_start(u0s[:, :], u0_d.ap())
            nc.sync.dma_start(u1s[:, :], u1_d.ap())
            w0m = cpool.tile([CINB, 5 * OC], HF, tag="w0b")
            u0m = cpool.tile([OC, 5 * OC], HF, tag="u0b")
            u1m = cpool.tile([OC, 5 * OC], HF, tag="u1b")
            nc.vector.tensor_copy(w0m[:, :], w0s[:, :])
            nc.vector.tensor_copy(u0m[:, :], u0s[:, :])
            nc.vector.tensor_copy(u1m[:, :], u1s[:, :])

            # ---- rings and state ----
            # x ring: 32 slots of SWQ cols; h ring: 32 slots of HWQ cols
            xr = [spool.tile([CINB, RING * SWQ], HF, tag=f"xr{s}",
                             name=f"xr{s}") for s in range(2)]
            hr = [spool.tile([OC, RING * HWQ], HF, tag=f"hr{s}",
                             name=f"hr{s}") for s in range(2)]
            cb = [[spool.tile([OC, HWQ], HF, tag=f"cb{s}{k}", name=f"cb{s}{k}")
                   for k in range(3)] for s in range(2)]
            for s in range(2):
                nc.vector.memset(hr[s][:, :], 0.0)
                for k in range(3):
                    nc.vector.memset(cb[s][k][:, :], 0.0)

            def load_x_chunk(s, c):
                """DMA x chunk c (steps [16c, 16c+16)) into its ring half."""
                d0 = c * CHUNK
                n = min(CHUNK, NSTEP - d0) * SWQ
                nc.sync.dma_start(
                    xr[s][:, (d0 % RING) * SWQ: (d0 % RING) * SWQ + n],
                    xs_d[s].ap()[:, d0 * SWQ: d0 * SWQ + n])

            def store_h_chunk(s, c):
                """DMA h chunk c (steps [16c, 16c+16)) from its ring half."""
                d0 = c * CHUNK
                n = min(CHUNK, NSTEP - d0) * HWQ
                nc.sync.dma_start(
                    outs_d[s].ap()[:, d0 * HWQ: d0 * HWQ + n],
                    hr[s][:, (d0 % RING) * HWQ: (d0 % RING) * HWQ + n])

            load_x_chunk(0, 0)
            load_x_chunk(1, 0)
            load_x_chunk(0, 1)
            load_x_chunk(1, 1)

            class Step:
                pass

            XORD = (J_L, J_F, J_G, J_I, J_O)
            UORD = (J_L, J_F, J_G, J_I, J_O)

            def mm_x_lf(s, d, st):
                """x-projection into the l,f bank (issued at step top; the
                lf bank's only reader sig_lf(d-1) finished early)."""
                y0, y1 = _window(d)
                nwin = (y1 - y0 + 1) * BQ
                st.y0, st.nwin = y0, nwin
                st.lo = BQ + y0 * BQ
                st.xv = xr[s][:, (d % RING) * SWQ + y0 * BQ:
                              (d % RING) * SWQ + y0 * BQ + nwin]
                # PSUM: one 2KB bank == one accumulation group (start=True
                # zeroes the whole bank).  l,f in one bank; i,g in another
                # (one sigmoid covers the ig pair); o alone, its sigmoid
                # deferred off the critical chain.
                st.lf = ppool.tile([OC, 2 * SWQ], FP, tag="lf", bufs=2,
                                   name=f"lf{s}_{d}")
                st.ig = ppool.tile([OC, 2 * SWQ], FP, tag="ig", bufs=2,
                                   name=f"ig{s}_{d}")
                st.og = ppool.tile([OC, SWQ], FP, tag="og", bufs=2,
                                   name=f"og{s}_{d}")

                def dst(j):
                    if j in (J_L, J_F):
                        return st.lf[:, (j - J_L) * nwin:(j - J_L + 1) * nwin]
                    if j == J_O:
                        return st.og[:, 0:nwin]
                    k = 0 if j == J_I else nwin
                    return st.ig[:, k:k + nwin]

                st.dst = dst
                for j in (J_L, J_F):
                    nc.tensor.matmul(dst(j),
                                     w0m[:, j * OC:(j + 1) * OC], st.xv,
                                     start=(j == J_L), stop=False)

            def mm_x_igo(s, d, st):
                """x-projection into the i,g and o banks, issued right
                before this sub-scan's u-matmuls: by then the banks'
                readers (sig_ig / sig_o of d-1) are long done, so the WAR
                wait that would otherwise head-of-line-block the PE FIFO
                is free."""
                for j in (J_I, J_G, J_O):
                    nc.tensor.matmul(st.dst(j),
                                     w0m[:, j * OC:(j + 1) * OC], st.xv,
                                     start=(j in (J_I, J_O)), stop=False)

            def mm_u(s, d, st):
                """h-recurrent matmuls for sub-scan s at step d."""
                nwin, lo = st.nwin, st.lo
                pbase = ((d - 1) % RING) * HWQ
                rhs_up = hr[s][:, pbase + lo - BQ: pbase + lo - BQ + nwin]
                rhs_lf = hr[s][:, pbase + lo: pbase + lo + nwin]
                stops = (J_F, J_G, J_O)
                for j in UORD:
                    nc.tensor.matmul(st.dst(j),
                                     u0m[:, j * OC:(j + 1) * OC], rhs_up,
                                     start=False, stop=False)
                    nc.tensor.matmul(st.dst(j),
                                     u1m[:, j * OC:(j + 1) * OC], rhs_lf,
                                     start=False, stop=(j in stops))

            def dcx_pre(s, d, st, prev):
                """dcx = c_up - c_lf: only needs the previous step's c (Pool,
                off the critical path)."""
                nwin, lo = st.nwin, st.lo
                c_up = cb[s][prev][:, lo - BQ: lo - BQ + nwin]
                c_lf = cb[s][prev][:, lo: lo + nwin]
                dcx = tpool.tile([OC, SWQ], HF, tag=f"dcx{s}", name=f"dcx{s}_{d}")
                nc.gpsimd.tensor_tensor(dcx[:, 0:nwin], c_up, c_lf, alu.subtract)
                st.dcx = dcx

            def sig_lf(s, d, st):
                """sigmoid over the l,f bank (fires after 4 u-matmuls)."""
                nwin = st.nwin
                g4 = gpool.tile([OC, 2 * SWQ], HF, tag=f"g4{s}", name=f"g4{s}_{d}")
                st.g4 = g4
                nc.scalar.activation(g4[:, 0:2 * nwin],
                                     st.lf[:, 0:2 * nwin], act.Sigmoid)

            def sig_ig(s, d, st):
                """one sigmoid over the i,g bank (g pre-scaled x2 on host)."""
                nwin = st.nwin
                gig = gpool.tile([OC, 2 * SWQ], HF, tag=f"gig{s}",
                                 name=f"gig{s}_{d}")
                st.gig = gig
                nc.scalar.activation(gig[:, 0:2 * nwin], st.ig[:, 0:2 * nwin],
                                     act.Sigmoid)

            def sig_o(s, d, st):
                """sigmoid over the o bank; only h needs it, so it rides
                the ACT bubbles off the critical chain."""
                nwin = st.nwin
                go = gpool.tile([OC, SWQ], HF, tag=f"go{s}", name=f"go{s}_{d}")
                st.go = go
                nc.scalar.activation(go[:, 0:nwin], st.og[:, 0:nwin],
                                     act.Sigmoid)

            def pw_mix(s, d, st, prev):
                """the l,f half of the c-chain (only needs sig_lf)."""
                nwin, lo = st.nwin, st.lo
                c_lf = cb[s][prev][:, lo: lo + nwin]
                g4 = st.g4
                l_ = g4[:, 0 * nwin:1 * nwin]
                f_ = g4[:, 1 * nwin:2 * nwin]
                mix = tpool.tile([OC, SWQ], HF, tag=f"mix{s}", name=f"mix{s}_{d}")
                nc.vector.tensor_tensor(mix[:, 0:nwin], l_, st.dcx[:, 0:nwin],
                                        alu.mult)
                nc.vector.tensor_tensor(mix[:, 0:nwin], mix[:, 0:nwin], c_lf,
                                        alu.add)
                nc.vector.tensor_tensor(mix[:, 0:nwin], f_, mix[:, 0:nwin],
                                        alu.mult)
                st.mix = mix

            def pw_ig(s, d, st, cur):
                """i*g and the c write (needs sig_ig)."""
                nwin, lo = st.nwin, st.lo
                i_ = st.gig[:, 0:nwin]
                sg = st.gig[:, nwin:2 * nwin]
                # tanh(g) = 2*sigmoid(2g) - 1, fused dual-op tensor_scalar
                gg = tpool.tile([OC, SWQ], HF, tag=f"gg{s}", name=f"gg{s}_{d}")
                nc.vector.tensor_scalar(gg[:, 0:nwin], sg, 2.0, 1.0,
                                        alu.mult, alu.subtract)
                ig = tpool.tile([OC, SWQ], HF, tag=f"ig{s}", name=f"ig{s}_{d}")
                nc.vector.tensor_tensor(ig[:, 0:nwin], i_, gg[:, 0:nwin], alu.mult)
                cw = cb[s][cur][:, lo: lo + nwin]
                nc.vector.tensor_tensor(cw, st.mix[:, 0:nwin], ig[:, 0:nwin],
                                        alu.add)
                st.cw = cw

            def tanh_c(s, d, st):
                th = tpool.tile([OC, SWQ], HF, tag=f"th{s}", name=f"th{s}_{d}")
                nc.scalar.activation(th[:, 0:st.nwin], st.cw, act.Tanh)
                st.th = th

            def h_mul(s, d, st):
                nwin, lo = st.nwin, st.lo
                o_ = st.go[:, 0:nwin]
                base = (d % RING) * HWQ
                hwv = hr[s][:, base + lo: base + lo + nwin]
                nc.vector.tensor_tensor(hwv, o_, st.th[:, 0:nwin], alu.mult)

            scr = ppool.tile([OC, 512], FP, tag="scratch", bufs=1,
                             name="scratch")

            def pe_fill(k):
                """Dummy matmuls on constant tiles: no deps, keep the PE
                clock ramped while it waits for the h chain."""
                for _ in range(k):
                    nc.tensor.matmul(scr[:, 0:512], u1m[:, 0:OC],
                                     u0m[:, 0:512], start=True, stop=True)

            for d in range(NSTEP):
                cur, prev = d % 3, (d + 2) % 3
                stA, stB = Step(), Step()
                mm_x_lf(0, d, stA)
                mm_x_lf(1, d, stB)
                dcx_pre(0, d, stA, prev)
                dcx_pre(1, d, stB, prev)
                pe_fill(4)
                mm_x_igo(1, d, stB)
                mm_u(1, d, stB)
                sig_lf(1, d, stB)
                if d > 0:
                    # A's tail from step d-1, deferred: its o-sig/tanh/h
                    # slot into the ACT/DVE queues here (deps long met)
                    # instead of head-of-line blocking this step's B burst
                    sig_o(0, d - 1, stA_prev)
                    tanh_c(0, d - 1, stA_prev)
                    h_mul(0, d - 1, stA_prev)
                    if (d - 1) % CHUNK == CHUNK - 1:
                        # A's chunk store waits for the deferred h above
                        store_h_chunk(0, (d - 1) // CHUNK)
                sig_ig(1, d, stB)
                pw_mix(1, d, stB, prev)
                pw_ig(1, d, stB, cur)
                mm_x_igo(0, d, stA)
                mm_u(0, d, stA)
                pe_fill(1)
                sig_o(1, d, stB)
                tanh_c(1, d, stB)         # ACT: B's tanh ahead of A's sigmoids
                sig_lf(0, d, stA)
                sig_ig(0, d, stA)
                pw_mix(0, d, stA, prev)
                pw_ig(0, d, stA, cur)
                h_mul(1, d, stB)
                stA_prev = stA
                if d % CHUNK == CHUNK - 1:
                    c = d // CHUNK
                    # B's h chunk done -> store (A's ships next step, after
                    # its deferred tail); prefetch x chunk c+2
                    store_h_chunk(1, c)
                    if (c + 2) * CHUNK < NSTEP:
                        load_x_chunk(0, c + 2)
                        load_x_chunk(1, c + 2)
            # flush A's deferred tail from the last step
            sig_o(0, NSTEP - 1, stA_prev)
            tanh_c(0, NSTEP - 1, stA_prev)
            h_mul(0, NSTEP - 1, stA_prev)
            # tail: steps 144..158 are chunk 9 (15 steps); A's chunk 8
            # store was deferred into step 144 above
            store_h_chunk(0, NSTEP // CHUNK)
            store_h_chunk(1, NSTEP // CHUNK)

    nc.compile()
    return nc


_NC_CACHE = {}


def _get_nc():
    if "nc" not in _NC_CACHE:
        _NC_CACHE["nc"] = build_kernel()
    return _NC_CACHE["nc"]


def _flip(x, d):
    if d == 1:
        return x[:, :, :, ::-1]
    if d == 2:
        return x[:, :, ::-1, :]
    if d == 3:
        return x[:, :, ::-1, ::-1]
    return x


def _make_x_diag(x_nat):
    """(CIN,H,W,BQ) -> (CINB, NSTEP*SWQ) diagonal layout with ones-rows."""
    arr = np.zeros((CINB, NSTEP, H, BQ), np.float32)
    for y in range(H):
        arr[:CIN, y:y + W, y, :] = x_nat[:, y, :, :]
    arr[CIN:, :, :, :] = 1.0
    return arr.reshape(CINB, NSTEP * SWQ)


def _decode(out_diag):
    """(OC, NSTEP*HWQ) fp32 -> (BQ, OC, H, W); skip the guard columns."""
    arr = out_diag.reshape(OC, NSTEP, HWQ)[:, :, BQ:].reshape(OC, NSTEP, H, BQ)
    out = np.empty((BQ, OC, H, W), np.float32)
    for y in range(H):
        out[:, :, y, :] = arr[:, y:y + W, y, :].transpose(2, 0, 1)
    return out


def kernel(x, w0, u0, u1, b, trace=False, _res=[None]):
    x = np.asarray(x, np.float32)
    w0 = np.asarray(w0, np.float32)
    u0 = np.asarray(u0, np.float32)
    u1 = np.asarray(u1, np.float32)
    b = np.asarray(b, np.float32)

    perm = np.concatenate([np.arange(g * OC, (g + 1) * OC) for g in GATE_ORDER])
    in_maps = []
    for c in range(8):
        dirn, half = c % 4, c // 4
        xs = _flip(x[half * 16:(half + 1) * 16], dirn)          # (16,CIN,H,W)
        x_nat = np.ascontiguousarray(xs.transpose(1, 2, 3, 0))  # (CIN,H,W,16)
        # g-gate pre-scale: tanh(g) = 2*sigmoid(2g) - 1, so double the g
        # columns of every projection (and bias) and apply sigmoid on-chip
        w0p = w0[dirn][:, perm].copy()
        u0p = u0[dirn][:, perm].copy()
        u1p = u1[dirn][:, perm].copy()
        bp = b[dirn][perm].copy()
        w0p[:, J_G * OC:(J_G + 1) * OC] *= 2.0
        u0p[:, J_G * OC:(J_G + 1) * OC] *= 2.0
        u1p[:, J_G * OC:(J_G + 1) * OC] *= 2.0
        bp[J_G * OC:(J_G + 1) * OC] *= 2.0
        # bias split into fp16 hi + residual lo rows so it lands fp32-accurate
        b_hi = bp.astype(np.float16).astype(np.float32)
        b_lo = bp - b_hi
        w0b = np.concatenate([w0p, b_hi[None, :], b_lo[None, :]], axis=0)
        m = {
            "w0": np.ascontiguousarray(w0b),
            "u0": np.ascontiguousarray(u0p),
            "u1": np.ascontiguousarray(u1p),
        }
        for s in range(2):
            m[f"x_diag{s}"] = _make_x_diag(
                x_nat[:, :, :, s * BQ:(s + 1) * BQ]).astype(np.float16)
        in_maps.append(m)

    nc = _get_nc()
    res = bass_utils.run_bass_kernel_spmd(nc, in_maps, list(range(8)), trace=trace)
    _res[0] = res

    out = np.empty((B, 4, OC, H, W), np.float32)
    for c in range(8):
        dirn, half = c % 4, c // 4
        for s in range(2):
            od = np.asarray(res.results[c][f"out_diag{s}"]).astype(np.float32)
            lo = half * 16 + s * BQ
            out[lo:lo + BQ, dirn] = _decode(od)
    return out

